# revision 16
# baseline (speedup 1.0000x reference)
"""Trainium2 Bass kernel: log-odds transform + uniform-grid histogram binning.

Reference semantics (f32, bins = jnp.linspace(-8, 8, 4096), Xs in
[1e-3, 1-1e-3]):
    s   = log(Xs) - log(1 - Xs)
    idx = clip(searchsorted(bins, max(s, bins[0]), side='right') - 1, 0, 4095)
    out = bins[idx]              # straight-through forward value

Design (hybrid column split, 4 engines)
---------------------------------------
The uniform grid turns searchsorted into arithmetic:
    k = round((ln(x) - ln(1-x))*invw + 2047)
Measured engine rates (ns/col, 128 lanes, fast-DVFS): ACT activation
0.97, DVE tensor_scalar 0.60, DVE 2-stream custom ~1.2-1.9, GpSimd TS
~1.9. DMA caps at ~435 GB/s/core -> 12.58MB of traffic = 28.9us
= 1.76 ns/col. A single engine can't carry both logs (2*0.97) nor can
the DVE carry the whole bit-trick path, so the second log is split by
columns (zf = 0.69):

  path A (cols [0:z]): ACT computes Ln(1-x) via the free input affine
      (scale=-1, bias=1); a 3-stage fused DVE op (AROUND) emits u16
      bins: k = RN((a - b)*invw + 2047)     [u16 convert rounds-to-
      nearest on HW, proven by probe]
  path B (cols [z:fd]): POOL materializes y = 1-x; an 8-stage custom
      DVE op (LOG2Q) reads y through BOTH an int32-bitcast AP (the DVE
      input converter yields t = float(bits(y)), i.e. exponent+mantissa
      as one number) and the f32 AP (BITWISE_AND with 1.0f isolates the
      exponent field, BITWISE_NOT flips it, y*~(y&1.0f) ~ -4m gives the
      exact mantissa), producing
      W = C*(i + C1*(y*~(y&1.0f) + C0)^2),  C = -ln2*invw*2^-23,
      a log2-linear term plus a general-quadratic minimax fit of
      log2(1+f)-f (|err| < 5e-3 log2-units ~ 0.9 bins); a 3-stage
      fused op (BCOMB) finishes k = RN(W + invw*a + D2).

Per 2048-col tile (fast mode): ACT ~3.35us, DVE ~3.3us, POOL ~1.1us vs
the 3.45us DMA cadence -> the kernel is DMA-bound in the input phase.
Accuracy: path A is the exact baseline arithmetic (2.6e-5 L2), path B
adds 1.5e-3 on 31% of elements -> total L2 rel err 8.3e-4 (gate 2e-2).

Output is u16 bin indices; the host expands through the caller's bins
table while unsharding. Tiles ramp 1024 -> 2048x6 -> 1536/1024/512 so
first compute starts ~1.2us after the first DMA and the drain tail is
short. Fixed NEFF overhead (engine bring-up + end-of-NEFF semaphore
teardown storm) measured at ~12.9us on this stack, invariant to kernel
structure; with the ~28.9us DMA floor the practical lower bound is
~42us. Measured: ~45-47us per core fast-mode (50.7us baseline).

Notes from rejected variants: GpSimd cannot run scalar_tensor_tensor
(not a Pool opcode) and its TensorTensor cannot dtype-convert f32->u16;
NBUF=7 (7*28KB/partition on top of the ~16.4KB framework base) crosses
the usable-SBUF boundary and silently corrupts; splitting the combine
into 4 DVE + 2 POOL ops per tile regresses ~10us (per-instruction
overhead / SBUF port contention).
"""

import numpy as np

import concourse.bacc as bacc
import concourse.mybir as mybir
from concourse import bass_utils
from concourse.mybir import AluOpType

# ---------------------------------------------------------------------------
# Custom DVE ops (runtime registration into concourse.dve_ops.OPS)
# ---------------------------------------------------------------------------
from concourse.dve_spec import (
    Spec, Src0, Src1, C0, C1, C2, One, lower, AluOp, Bin,
)
from concourse.dve_spec import _has_src1 as has_src1
from concourse import dve_ops as DO
from concourse.dve_uop import DveOpSpec


def _register_dve_op(name, spec, subdim=False):
    if name in DO._SUB_OPCODE_FOR_NAME:
        for op in DO.OPS:
            if op.name == name:
                return op
        raise RuntimeError(f"{name} registered but not in OPS")
    row = DO._CUSTOM_DVE_ROW_BASE + len(DO.OPS)
    assert row < 0x20, "custom DVE row space exhausted"
    DO._SUB_OPCODE_FOR_NAME[name] = row
    shas = {}
    for ver in ("v3", "v4"):
        uops = lower(spec, ver=ver)
        shas[ver] = DveOpSpec(
            name=name, opcode=row, uops=uops, rd1_en=has_src1(spec)
        ).sha(ver)
    op = DO.DveOp(name, spec, subdim=subdim, uops_sha=shas)
    DO.OPS.append(op)
    DO.CUSTOM_DVE_SPECS[name] = spec
    return op


def _log2q_ref(in0, in1, s0, s1, imm2):
    i = in0.astype(np.float32)
    E = in1.view(np.int32) & np.int32(0x3F800000)
    nE = (~E).view(np.float32)
    mm = (in1 * nE).astype(np.float32)
    h2 = (mm + np.float32(s0)).astype(np.float32)
    V = (i + ((h2 * h2) * np.float32(s1)).astype(np.float32)).astype(np.float32)
    return (V * np.float32(imm2)).astype(np.float32)


_E = Bin(AluOp.BITWISE_AND, Src1, One)
_nE = Bin(AluOp.BITWISE_NOT, _E, _E)
_h2 = (Src1 * _nE) + C0
LOG2Q = _register_dve_op(
    "LOG2Q_ANT", Spec(body=(Src0 + (_h2 * _h2) * C1) * C2, reference=_log2q_ref)
)

# k = (a - b)*invw + 2047  (u16 output converter rounds to nearest)
AROUND = _register_dve_op(
    "AROUND_ANT",
    Spec(
        body=(Src0 - Src1) * C0 + C1,
        reference=lambda in0, in1, s0, s1, imm2: (
            ((in0.astype(np.float32) - in1) * np.float32(s0)).astype(np.float32)
            + np.float32(s1)
        ).astype(np.float32),
    ),
)

# k = W + a*invw + D2
BCOMB = _register_dve_op(
    "BCOMB_ANT",
    Spec(
        body=(Src0 + Src1 * C0) + C1,
        reference=lambda in0, in1, s0, s1, imm2: (
            (in0.astype(np.float32) + (in1 * np.float32(s0)).astype(np.float32))
            + np.float32(s1)
        ).astype(np.float32),
    ),
)

# ---------------------------------------------------------------------------
N = 16_777_216
NCORES = 8
SHARD = N // NCORES
P = 128
COLS = SHARD // P                     # 16384
NUM_BINS = 4096

F32 = mybir.dt.float32
I32 = mybir.dt.int32
U16 = mybir.dt.uint16
Ln = mybir.ActivationFunctionType.Ln

INVW = float(np.float32(4095.0 / 16.0))
G = float(np.log(2.0) * np.float64(np.float32(4095.0 / 16.0)))
C_STT = float(np.float32(-G * 2.0**-23))      # log2->bin scale (path B)
A_C0 = 5.90466605                              # quadratic center
A_C1 = -176045.285                             # quadratic scale
D2 = 24561.9725                                # path-B offset
CADD = 2047.0                                  # path-A offset

FDS = [1024] + [2048] * 6 + [1536, 1024, 512]
assert sum(FDS) == COLS
NBUF = 6
FDMAX = max(FDS)
ZF = 0.69                                      # ACT share of the 2nd log


def _split(fd, frac):
    c = int(round(fd * frac / 64.0)) * 64
    return min(max(c, 64), fd - 64) if fd >= 128 else fd


def build_module(fds=None, nbuf=NBUF, zf=ZF):
    fds = list(FDS if fds is None else fds)
    assert sum(fds) == COLS
    nt = len(fds)
    starts = np.concatenate([[0], np.cumsum(fds)]).astype(int)

    nc = bacc.Bacc("TRN2", target_bir_lowering=False, debug=False)
    x = nc.dram_tensor("x", [SHARD], F32, kind="ExternalInput")
    y = nc.dram_tensor("y", [SHARD], U16, kind="ExternalOutput")
    xf, yf = x[:], y[:]

    def tile_ap(base, i):
        s, fd = int(starts[i]) * P, fds[i]
        return base[s:s + fd * P].rearrange("(p m) -> p m", p=P, m=fd)

    with (
        nc.sbuf_tensor("xb", [P, nbuf * FDMAX], F32) as xb,
        nc.sbuf_tensor("ab", [P, nbuf * FDMAX], F32) as ab,
        nc.sbuf_tensor("cb", [P, nbuf * FDMAX], F32) as cb,
        nc.sbuf_tensor("ob", [P, nbuf * FDMAX], U16) as ob,
        nc.sbuf_tensor("warm", [P, 1], F32) as warm,
        nc.semaphore("in_sem") as in_sem,      # +16 per DMA-in
        nc.semaphore("act_sem") as act_sem,    # +2/tile: Ln, Ln(1-x)
        nc.semaphore("pool_sem") as pool_sem,  # +1/tile: y
        nc.semaphore("vec_sem") as vec_sem,    # +3/tile: W, kB, kA
        nc.semaphore("out_sem") as out_sem,    # +16 per DMA-out
        nc.Block() as block,
    ):
        def sl(buf, i, lo, hi, dt=None):
            s = (i % nbuf) * FDMAX
            ap = buf[:, s + lo:s + hi]
            return ap.bitcast(dt) if dt is not None else ap

        za = [_split(fd, zf) for fd in fds]    # path-A cols [0:z)

        @block.sync
        def _(sync):
            for i in range(min(nbuf, nt)):
                sync.dma_start(sl(xb, i, 0, fds[i]), tile_ap(xf, i)).then_inc(
                    in_sem, 16
                )
            for i in range(nt):
                if i + nbuf < nt:
                    sync.wait_ge(act_sem, 2 * i + 2)
                    sync.wait_ge(pool_sem, i + 1)
                    sync.dma_start(
                        sl(xb, i + nbuf, 0, fds[i + nbuf]),
                        tile_ap(xf, i + nbuf),
                    ).then_inc(in_sem, 16)
                sync.wait_ge(vec_sem, 3 * i + 3)
                sync.dma_start(tile_ap(yf, i), sl(ob, i, 0, fds[i])).then_inc(
                    out_sem, 16
                )
            sync.wait_ge(out_sem, 16 * nt)
            sync.sem_clear(in_sem)
            sync.sem_clear(act_sem)
            sync.sem_clear(pool_sem)
            sync.sem_clear(vec_sem)
            sync.sem_clear(out_sem)

        @block.scalar
        def _(scalar):
            # issue an Ln before any data wait: ACT_TABLE_LOAD for the
            # natural_log set overlaps the first DMA.
            nc.scalar.activation(warm[:, :], warm[:, :], Ln)
            for i in range(nt):
                fd, z = fds[i], za[i]
                scalar.wait_ge(in_sem, 16 * (i + 1))
                if i >= nbuf:
                    scalar.wait_ge(vec_sem, 3 * (i - nbuf) + 3)
                nc.scalar.activation(
                    sl(ab, i, 0, fd), sl(xb, i, 0, fd), Ln
                ).then_inc(act_sem, 1)
                nc.scalar.activation(
                    sl(cb, i, 0, z), sl(xb, i, 0, z), Ln,
                    bias=1.0, scale=-1.0,
                ).then_inc(act_sem, 1)

        @block.gpsimd
        def _(gpsimd):
            for i in range(nt):
                fd, z = fds[i], za[i]
                gpsimd.wait_ge(in_sem, 16 * (i + 1))
                if i >= nbuf:
                    gpsimd.wait_ge(vec_sem, 3 * (i - nbuf) + 3)
                nc.gpsimd.tensor_scalar(
                    sl(cb, i, z, fd), sl(xb, i, z, fd), -1.0, 1.0,
                    AluOpType.mult, AluOpType.add,
                ).then_inc(pool_sem, 1)

        @block.vector
        def _(vector):
            for i in range(nt):
                fd, z = fds[i], za[i]
                vector.wait_ge(pool_sem, i + 1)
                nc.vector._custom_dve(
                    LOG2Q,
                    out=sl(cb, i, z, fd),
                    in0=sl(cb, i, z, fd, I32),
                    in1=sl(cb, i, z, fd),
                    s0=A_C0, s1=A_C1, imm2=C_STT,
                ).then_inc(vec_sem, 1)
                vector.wait_ge(act_sem, 2 * i + 1)
                if i >= nbuf:
                    vector.wait_ge(out_sem, 16 * (i - nbuf + 1))
                nc.vector._custom_dve(
                    BCOMB,
                    out=sl(ob, i, z, fd),
                    in0=sl(cb, i, z, fd),
                    in1=sl(ab, i, z, fd),
                    s0=INVW, s1=D2,
                ).then_inc(vec_sem, 1)
                vector.wait_ge(act_sem, 2 * i + 2)
                nc.vector._custom_dve(
                    AROUND,
                    out=sl(ob, i, 0, z),
                    in0=sl(ab, i, 0, z),
                    in1=sl(cb, i, 0, z),
                    s0=INVW, s1=CADD,
                ).then_inc(vec_sem, 1)

    nc.compile()
    return nc


_module_cache = {}


def _get_module(**kwargs):
    key = repr(sorted(kwargs.items()))
    if key not in _module_cache:
        _module_cache[key] = build_module(**kwargs)
    return _module_cache[key]


def run(Xs, bins, trace=False, **build_kwargs):
    Xs = np.ascontiguousarray(np.asarray(Xs, dtype=np.float32))
    assert Xs.shape == (N,), Xs.shape
    bins_np = np.asarray(bins, dtype=np.float32)
    nc = _get_module(**build_kwargs)
    shards = Xs.reshape(NCORES, SHARD)
    in_maps = [{"x": shards[c]} for c in range(NCORES)]
    res = bass_utils.run_bass_kernel_spmd(
        nc, in_maps, core_ids=list(range(NCORES)), trace=trace
    )
    raw = np.concatenate([np.asarray(r["y"]) for r in res.results])
    out = np.take(bins_np, np.minimum(raw, NUM_BINS - 1).astype(np.int64))
    return out.astype(np.float32), res


def kernel(Xs, bins):
    out, _ = run(Xs, bins)
    return out
